# revision 9
# baseline (speedup 1.0000x reference)
"""Sparse 3D conv (gather -> per-offset GEMM -> scatter-add over K) on 8 trn2 cores.

Computation: out[m, o] = sum_k sum_c feats[in_idx[k, m], c] * mask[k, m] * kernel[k, c, o]

Strategy (per core, output voxels sharded 8 ways, 25 tiles of 512):
  Two-stage gather that avoids the ~1us-per-instruction Q7 descriptor-gen
  serialization of indirect_dma_start (the old baseline needed 2800 of them):

  - Host: feats packed as feats256 [200001, 128] fp16 where row 1+v = feats[v]
    tiled 4x (256B rows); row 0 = zeros (target for masked slots). Per tile,
    the ~6.8K unique active voxel ids are sorted and split into 7 windows of
    32768 ids (int16 index limit), each padded to a static 1408 slots.
  - Stage A: 7 dma_gather(transpose=True) per tile — one SWDGE instruction
    gathers an entire window segment HBM->SBUF into compact_T[128, NPOS]
    (channel-on-partition, 4 replicas; column = compact position).
  - Stage B: 7 indirect_copy per tile — per-16-partition-group uint16 offsets
    expand compact positions to the (k-slot, column) matmul grid [128, 512].
    Masked slots read position 0 (the zero row): mask multiply is free.
  - 7 accumulating matmuls (contraction 128 = 4 k-slots x 32 ch) into
    PSUM [64, 512] f32, copy to SBUF, DMA to out_t [64, M] (transposed).
"""

import numpy as np

N_VOX = 200000
M_VOX = 100000
K_VOL = 27
C_IN = 32
C_OUT = 64
N_CORES = 8
M_CORE = M_VOX // N_CORES        # 12500
TILE_M = 512
N_TILES = (M_CORE + TILE_M - 1) // TILE_M  # 25
N_G = 7                          # groups of 4 k-offsets (28 slots, last padded)
N_SLOT = 28
NROWS = N_VOX + 1                # feats256 rows (row 0 = zeros)
W_WIN = 32768                    # int16-addressable window
N_WIN = (NROWS + W_WIN - 1) // W_WIN  # 7
CAP_WIN = 1408                   # static per-(tile,window) capacity (%128==0)
NPOS = N_WIN * CAP_WIN           # 9856 compact positions per tile
GCOLS = NPOS // 16               # stage-A idx cols per tile (int16, wrapped)
CCOLS = N_G * (TILE_M // 16)     # stage-B idx cols per tile (uint16, wrapped)


def _build_program(n_tiles=N_TILES, dbg=False):
    import concourse.tile as tile
    import concourse.mybir as mybir
    from concourse import bacc
    from concourse._compat import get_trn_type

    nc = bacc.Bacc(get_trn_type() or "TRN2", target_bir_lowering=False, debug=False,
                   num_swdge_queues=4)

    feats_h = nc.dram_tensor(
        "feats256", (NROWS, 128), mybir.dt.float16, kind="ExternalInput")
    if dbg:
        ct_h = nc.dram_tensor(
            "ct_dbg", (128, n_tiles * NPOS), mybir.dt.float16,
            kind="ExternalOutput")
        tb_h = nc.dram_tensor(
            "tb_dbg", (128, n_tiles * N_G * TILE_M), mybir.dt.float16,
            kind="ExternalOutput")
    gidx_h = nc.dram_tensor(
        "gidx", (128, n_tiles * GCOLS), mybir.dt.int16, kind="ExternalInput")
    cidx_h = nc.dram_tensor(
        "cidx", (128, n_tiles * CCOLS), mybir.dt.uint16, kind="ExternalInput")
    w_h = nc.dram_tensor(
        "w_sb", (128, N_G * C_OUT), mybir.dt.float16, kind="ExternalInput")
    out_h = nc.dram_tensor(
        "out_t", (C_OUT, n_tiles * TILE_M), mybir.dt.float32, kind="ExternalOutput")

    with tile.TileContext(nc) as tc:
        with (
            tc.tile_pool(name="const", bufs=1) as const,
            tc.tile_pool(name="ctp", bufs=3) as ctp,
            tc.tile_pool(name="tbp", bufs=4) as tbp,
            tc.tile_pool(name="psum", bufs=4, space="PSUM") as psum,
            tc.tile_pool(name="outp", bufs=3) as outp,
        ):
            gidx_sb = const.tile([128, n_tiles * GCOLS], mybir.dt.int16)
            nc.sync.dma_start(gidx_sb[:], gidx_h[:])
            cidx_sb = const.tile([128, n_tiles * CCOLS], mybir.dt.uint16)
            nc.sync.dma_start(cidx_sb[:], cidx_h[:])
            w_sb = const.tile([128, N_G * C_OUT], mybir.dt.float16)
            nc.sync.dma_start(w_sb[:], w_h[:])

            # global Pool-DMA counter: the tile scheduler assigns DMASW sem
            # lanes round-robin (mod 8) in program order and each lane is
            # runtime-locked to one SWDGE queue, so queue must be i%4 with i
            # counting ALL Pool DMAs program-wide.
            qn = 0
            for t in range(n_tiles):
                ct = ctp.tile([128, NPOS], mybir.dt.float16, tag="ct")
                for w in range(N_WIN):
                    lo = w * W_WIN
                    hi = min(lo + W_WIN, NROWS)
                    # SWDGE ring holds 1024 descriptors incl. 16 sem-incs:
                    # >1008 valid idxs wedges the device. Chunk at 896 (%128).
                    for s in range(0, CAP_WIN, 896):
                        n = min(896, CAP_WIN - s)
                        p0 = w * CAP_WIN + s
                        nc.gpsimd.dma_gather(
                            out_ap=ct[:, p0:p0 + n].unsqueeze(1),
                            in_ap=feats_h[lo:hi, :],
                            idxs_ap=gidx_sb[
                                :, t * GCOLS + p0 // 16:
                                t * GCOLS + (p0 + n) // 16],
                            num_idxs=n,
                            num_idxs_reg=n,
                            elem_size=128,
                            transpose=True,
                            queue_num=0,
                        )
                        qn += 1
                if dbg:
                    nc.sync.dma_start(ct_h[:, t * NPOS:(t + 1) * NPOS], ct[:])
                ps = psum.tile([C_OUT, TILE_M], mybir.dt.float32, tag="ps")
                for g in range(N_G):
                    tb = tbp.tile([128, TILE_M], mybir.dt.float16, tag="tb")
                    nc.gpsimd.indirect_copy(
                        out=tb[:], data=ct[:],
                        idxs=cidx_sb[:, (t * N_G + g) * 32:(t * N_G + g + 1) * 32],
                        i_know_ap_gather_is_preferred=True)
                    if dbg:
                        nc.sync.dma_start(
                            tb_h[:, (t * N_G + g) * TILE_M:
                                 (t * N_G + g + 1) * TILE_M], tb[:])
                    nc.tensor.matmul(
                        out=ps[:],
                        lhsT=w_sb[:, g * C_OUT:(g + 1) * C_OUT],
                        rhs=tb[:],
                        start=(g == 0),
                        stop=(g == N_G - 1),
                    )
                ob = outp.tile([C_OUT, TILE_M], mybir.dt.float32, tag="ob")
                nc.scalar.copy(ob[:], ps[:])
                nc.sync.dma_start(out_h[:, t * TILE_M:(t + 1) * TILE_M], ob[:])

    nc.compile()
    return nc


def _wrap16(flat):
    """idx i -> [i % 16, i // 16]; returns [16, len/16]."""
    return np.ascontiguousarray(flat.reshape(-1, 16).T)


def pack_inputs(feats, kernel, in_idx, mask):
    """Host-side packing. Returns (feats256, per-core gidx, per-core cidx, w_sb)."""
    feats = np.asarray(feats, np.float32)
    kernel = np.asarray(kernel, np.float32)
    in_idx = np.asarray(in_idx)
    mask = np.asarray(mask)

    feats256 = np.zeros((NROWS, 128), np.float16)
    feats256[1:, :] = np.tile(feats.astype(np.float16), (1, 4))

    eidx = np.where(mask != 0, in_idx + 1, 0).astype(np.int64)  # [27, M]

    gidx_cores, cidx_cores = [], []
    for core in range(N_CORES):
        sl = eidx[:, core * M_CORE:(core + 1) * M_CORE]
        gidx = np.zeros((128, N_TILES * GCOLS), np.int16)
        cidx = np.zeros((128, N_TILES * CCOLS), np.uint16)
        for t in range(N_TILES):
            c0, c1 = t * TILE_M, min((t + 1) * TILE_M, M_CORE)
            S = np.zeros((N_SLOT, TILE_M), np.int64)
            S[:K_VOL, :c1 - c0] = sl[:, c0:c1]
            U = np.unique(S)                      # sorted, includes 0
            P = np.zeros(S.shape, np.int64)       # compact position per slot
            ga = np.zeros((N_WIN, CAP_WIN), np.int16)
            wa = S // W_WIN
            for w in range(N_WIN):
                seg = U[(U >= w * W_WIN) & (U < (w + 1) * W_WIN)] - w * W_WIN
                assert len(seg) <= CAP_WIN, (core, t, w, len(seg))
                ga[w, :len(seg)] = seg.astype(np.int16)
                m = wa == w
                if m.any():
                    P[m] = w * CAP_WIN + np.searchsorted(seg, (S - w * W_WIN)[m])
            gidx[:16, t * GCOLS:(t + 1) * GCOLS] = _wrap16(ga.reshape(-1))
            for g in range(N_G):
                blk = np.zeros((128, 32), np.uint16)
                for gr in range(8):
                    blk[16 * gr:16 * (gr + 1)] = _wrap16(
                        P[4 * g + gr // 2].astype(np.uint16))
                cidx[:, (t * N_G + g) * 32:(t * N_G + g + 1) * 32] = blk
        gidx[16:, :] = np.tile(gidx[:16, :], (7, 1))
        gidx_cores.append(np.ascontiguousarray(gidx))
        cidx_cores.append(np.ascontiguousarray(cidx))

    kpad = np.zeros((N_G * 4, C_IN, C_OUT), np.float32)
    kpad[:K_VOL] = kernel
    w_sb = np.transpose(
        kpad.reshape(N_G, 4, C_IN, C_OUT), (1, 2, 0, 3)).reshape(128, N_G * C_OUT)
    return feats256, gidx_cores, cidx_cores, np.ascontiguousarray(
        w_sb.astype(np.float16))


_NC_CACHE = {}


def get_program(n_tiles=N_TILES):
    if n_tiles not in _NC_CACHE:
        _NC_CACHE[n_tiles] = _build_program(n_tiles)
    return _NC_CACHE[n_tiles]


def run_on_device(feats256, gidx_cores, cidx_cores, w_sb, trace=False, tmpdir=None):
    from concourse import bass_utils
    from concourse.bass_interp import get_hw_module

    nc = get_program()
    in_maps = [
        {"feats256": feats256, "gidx": gidx_cores[c], "cidx": cidx_cores[c],
         "w_sb": w_sb}
        for c in range(N_CORES)
    ]
    old_m = nc.m
    nc.m = get_hw_module(nc.m)
    try:
        res = bass_utils.run_bass_kernel_spmd(
            nc, in_maps, core_ids=list(range(N_CORES)), trace=trace,
            tmpdir=tmpdir)
    finally:
        nc.m = old_m
    return res


def kernel(feats, kernel, in_idx, mask):
    feats256, gidx_cores, cidx_cores, w_sb = pack_inputs(feats, kernel, in_idx, mask)
    res = run_on_device(feats256, gidx_cores, cidx_cores, w_sb)
    outs = [res.results[c]["out_t"][:, :M_CORE].T for c in range(N_CORES)]
    return np.ascontiguousarray(np.concatenate(outs, 0), dtype=np.float32)
